# revision 18
# baseline (speedup 1.0000x reference)
"""MinkowskiInstanceNorm (segment-reduce instance norm) on 8 Trainium2 cores.

Strategy: seg_ids are sorted, so each segment is a contiguous run of rows.
With num_segments == n_cores == 8, core j owns segment j outright: it
computes sum(x) and sum(x^2) over its rows (padded to a fixed tile count
with zeros so one SPMD program serves all cores), derives
mean / inv_std / affine on-device, and normalizes in a second streaming
pass.  No cross-core communication is needed; the host only slices rows
per segment and stitches the outputs back in order.

Layout trick: the host packs each core's slab with CHANNELS ON
PARTITIONS — partition p = rb*32 + c (rb = row-block 0..3, c = channel),
free axis = 2048 consecutive rows of that block, i.e. x[T, 128, 2048].
Consequences on-device, per 1 MiB tile:
  - per-channel sums are plain full-free reductions ([128,2048]->[128,1]):
    one DVE tensor_reduce for sum(x), one ACT Square-activation whose
    accum_out gives sum(x^2) for free;
  - the normalization is ONE single-src DVE tensor_scalar
    (out = x*A[p] + B[p]) with per-partition scalars;
  - cross-partition folding (4 row-blocks per channel) is a tiny
    [128]x[128,2] matmul against a 0/1 selector, and the A/B broadcast
    back to 128 partitions is the transposed selector matmul.
PE and GpSimd stay idle; DVE+ACT total ~40% of the DMA streaming time,
so the kernel is HBM-bound as the problem demands.
"""

from contextlib import ExitStack

import numpy as np

C = 32  # channels
P = 128  # SBUF partitions
RB = P // C  # row blocks per tile (4)
FD = 2048  # rows per partition per tile (free dim)
ROWS = RB * FD  # rows per tile (8192)
NCORES = 8
EPS = 1e-8

_PROGRAMS = {}


def _emit(nc, tc, ctx, x_d, invn_d, w_d, b_d, s128_d, s32_d, o_d, T):
    from concourse import mybir

    dt = mybir.dt
    AX = mybir.AxisListType
    OP = mybir.AluOpType
    AF = mybir.ActivationFunctionType

    xv = x_d.ap()  # [T, P, FD]
    ov = o_d.ap()

    const = ctx.enter_context(tc.tile_pool(name="const", bufs=1))
    xpool = ctx.enter_context(tc.tile_pool(name="xpool", bufs=3))
    sqpool = ctx.enter_context(tc.tile_pool(name="sqpool", bufs=1))
    ypool = ctx.enter_context(tc.tile_pool(name="ypool", bufs=2))
    opool = ctx.enter_context(tc.tile_pool(name="opool", bufs=2))
    psum = ctx.enter_context(tc.tile_pool(name="psum", bufs=1, space="PSUM"))

    # First RES tiles stay resident in SBUF across both passes, skipping
    # their pass-2 reload (saves RES MiB of HBM reads per core).
    RES = min(15, T)
    res = const.tile([P, RES * FD], dt.float32)
    # single scratch for the (unread) Square outputs; ACT serializes anyway
    sqscratch = sqpool.tile([P, FD], dt.float32)

    invn = const.tile([C, 1], dt.float32)
    nc.sync.dma_start(out=invn[:], in_=invn_d.ap())
    wt = const.tile([C, 1], dt.float32)
    nc.sync.dma_start(out=wt[:], in_=w_d.ap())
    bt = const.tile([C, 1], dt.float32)
    nc.sync.dma_start(out=bt[:], in_=b_d.ap())
    sel128 = const.tile([P, C], dt.float32)
    nc.sync.dma_start(out=sel128[:], in_=s128_d.ap())
    sel32 = const.tile([C, P], dt.float32)
    nc.sync.dma_start(out=sel32[:], in_=s32_d.ap())

    sparts = const.tile([P, T], dt.float32)
    qparts = const.tile([P, T], dt.float32)

    for i in range(T):
        if i < RES:
            xt = res[:, i * FD : (i + 1) * FD]
        else:
            xt = xpool.tile([P, FD], dt.float32, tag="xt")
        nc.sync.dma_start(out=xt[:], in_=xv[i])
        nc.vector.tensor_reduce(
            out=sparts[:, i : i + 1], in_=xt[:], axis=AX.X, op=OP.add
        )
        nc.scalar.activation(
            sqscratch[:], xt[:], AF.Square, accum_out=qparts[:, i : i + 1]
        )

    st2 = const.tile([P, 2], dt.float32)
    nc.vector.tensor_reduce(out=st2[:, 0:1], in_=sparts[:], axis=AX.X, op=OP.add)
    nc.vector.tensor_reduce(out=st2[:, 1:2], in_=qparts[:], axis=AX.X, op=OP.add)

    # fold the RB row-blocks of each channel: [32, 2] = sel128.T @ st2
    tot = psum.tile([C, 2], dt.float32)
    nc.tensor.matmul(tot[:], lhsT=sel128[:], rhs=st2[:], start=True, stop=True)

    mean = const.tile([C, 1], dt.float32)
    nc.vector.tensor_scalar_mul(mean[:], tot[:, 0:1], invn[:])
    ex2 = const.tile([C, 1], dt.float32)
    nc.vector.tensor_scalar_mul(ex2[:], tot[:, 1:2], invn[:])
    msq = const.tile([C, 1], dt.float32)
    nc.vector.tensor_mul(msq[:], mean[:], mean[:])
    var = const.tile([C, 1], dt.float32)
    nc.vector.tensor_sub(var[:], ex2[:], msq[:])
    epsv = const.tile([C, 1], dt.float32)
    nc.vector.memset(epsv[:], EPS)
    std = const.tile([C, 1], dt.float32)
    nc.scalar.activation(std[:], var[:], AF.Sqrt, bias=epsv[:])
    istd = const.tile([C, 1], dt.float32)
    nc.vector.reciprocal(istd[:], std[:])
    # ab = [A | B]: A = w/std, B = b - mean*A
    ab = const.tile([C, 2], dt.float32)
    nc.vector.tensor_mul(ab[:, 0:1], istd[:], wt[:])
    nc.vector.tensor_mul(ab[:, 1:2], mean[:], ab[:, 0:1])
    nc.vector.tensor_sub(ab[:, 1:2], bt[:], ab[:, 1:2])

    # broadcast A/B back to all 128 partitions: [128, 2] = sel32.T @ ab
    abps = psum.tile([P, 2], dt.float32)
    nc.tensor.matmul(abps[:], lhsT=sel32[:], rhs=ab[:], start=True, stop=True)
    ab128 = const.tile([P, 2], dt.float32)
    nc.scalar.copy(ab128[:], abps[:])

    for i in range(T):
        if i < RES:
            yt = res[:, i * FD : (i + 1) * FD]
        else:
            yt = ypool.tile([P, FD], dt.float32, tag="yt")
            nc.sync.dma_start(out=yt[:], in_=xv[i])
        ot = opool.tile([P, FD], dt.float32, tag="ot")
        nc.vector.tensor_scalar(
            out=ot[:],
            in0=yt[:],
            scalar1=ab128[:, 0:1],
            scalar2=ab128[:, 1:2],
            op0=OP.mult,
            op1=OP.add,
        )
        # stores ride the ACT HW-DGE ring so a store waiting on compute
        # never head-of-line-blocks the load FIFO on the sync ring
        nc.scalar.dma_start(out=ov[i], in_=ot[:])


def _get_program(T):
    if T in _PROGRAMS:
        return _PROGRAMS[T]
    import concourse.tile as tile
    from concourse import bacc, mybir

    dt = mybir.dt
    nc = bacc.Bacc(
        "TRN2",
        target_bir_lowering=False,
        debug=False,
        enable_asserts=False,
        num_devices=NCORES,
    )
    x_d = nc.dram_tensor("x", [T, P, FD], dt.float32, kind="ExternalInput")
    invn_d = nc.dram_tensor("invn", [C, 1], dt.float32, kind="ExternalInput")
    w_d = nc.dram_tensor("w", [C, 1], dt.float32, kind="ExternalInput")
    b_d = nc.dram_tensor("b", [C, 1], dt.float32, kind="ExternalInput")
    s128_d = nc.dram_tensor("sel128", [P, C], dt.float32, kind="ExternalInput")
    s32_d = nc.dram_tensor("sel32", [C, P], dt.float32, kind="ExternalInput")
    o_d = nc.dram_tensor("o", [T, P, FD], dt.float32, kind="ExternalOutput")

    with tile.TileContext(nc) as tc:
        with ExitStack() as ctx:
            _emit(nc, tc, ctx, x_d, invn_d, w_d, b_d, s128_d, s32_d, o_d, T)

    nc.finalize()
    _PROGRAMS[T] = nc
    return nc


def _pack(rows, T):
    """rows [n, C] -> [T, 128, FD]: partition rb*32+c holds rows
    [t*ROWS + rb*FD + j] of channel c at free index j; zero padded."""
    PAD = T * ROWS
    xp = np.zeros((PAD, C), dtype=np.float32)
    xp[: rows.shape[0]] = rows
    return np.ascontiguousarray(
        xp.reshape(T, RB, FD, C).transpose(0, 1, 3, 2).reshape(T, P, FD)
    )


def _unpack(slab, n):
    """[T, 128, FD] -> rows [n, C]."""
    T = slab.shape[0]
    return (
        slab.reshape(T, RB, C, FD).transpose(0, 1, 3, 2).reshape(T * ROWS, C)[:n]
    )


def kernel(feats, seg_ids, weight, bias, num_segments, **_):
    from concourse.bass_utils import run_bass_kernel_spmd

    feats = np.ascontiguousarray(np.asarray(feats), dtype=np.float32)
    seg = np.asarray(seg_ids)
    w = np.asarray(weight, dtype=np.float32).reshape(C, 1)
    b = np.asarray(bias, dtype=np.float32).reshape(C, 1)
    S = int(num_segments)
    N = feats.shape[0]

    assert (np.diff(seg) >= 0).all(), "seg_ids must be sorted"
    bounds = np.searchsorted(seg, np.arange(S + 1)).astype(np.int64)
    counts = np.diff(bounds)

    sel128 = np.ascontiguousarray(np.tile(np.eye(C, dtype=np.float32), (RB, 1)))
    sel32 = np.ascontiguousarray(sel128.T)

    out = np.empty((N, C), dtype=np.float32)
    for g0 in range(0, S, NCORES):
        gsegs = list(range(g0, min(g0 + NCORES, S)))
        maxc = max(int(counts[s]) for s in gsegs)
        T = max(1, -(-maxc // ROWS))
        nc = _get_program(T)
        in_maps = []
        for j in range(NCORES):
            n_j = 1
            if j < len(gsegs):
                s = gsegs[j]
                n_j = max(int(counts[s]), 1)
                rows = feats[bounds[s] : bounds[s + 1]]
            else:
                rows = np.zeros((0, C), dtype=np.float32)
            in_maps.append(
                {
                    "x": _pack(rows, T),
                    "invn": np.full((C, 1), 1.0 / n_j, dtype=np.float32),
                    "w": w,
                    "b": b,
                    "sel128": sel128,
                    "sel32": sel32,
                }
            )
        results = run_bass_kernel_spmd(nc, in_maps, list(range(NCORES))).results
        for j, s in enumerate(gsegs):
            out[bounds[s] : bounds[s + 1]] = _unpack(results[j]["o"], int(counts[s]))
    return out
